# revision 17
# baseline (speedup 1.0000x reference)
"""Trainium2 Bass kernel for nn_DecLayer (GNN message-passing decoder layer).

Reference computation per node n (B*N = 8192 nodes, K = 48 neighbors):
    h_EV[k]   = concat(h_V[n], h_E[n,k])                    # [K, 512]
    m1[k]     = gelu(h_EV[k] @ W1 + b1)                     # [K, 128]
    m2[k]     = gelu(m1[k] @ W2 + b2)
    m3[k]     = mask_attend[n,k] * (m2[k] @ W3 + b3)
    dh        = sum_k m3[k] / 30
    h         = LN(alpha*dh + h_V[n]; g1, be1)
    dh2       = gelu(h @ Wi + bi) @ Wo + bo
    h         = LN(alpha*dh2 + h; g2, be2)
    out       = mask_V[n] * h

Strategy (8 cores, data-parallel over nodes, 1024 nodes/core):
  - On-chip activations live TRANSPOSED: [feature(128 partitions), item(free)].
  - h_E is loaded in k-major windows (512 rows = one k for a chunk of 512
    nodes), PE-transposed 128x128 tile-wise into X^T (fp16 by default), and
    consumed by the W1 matmuls.
  - h_V's contribution to layer 1 is re-added per window with one matmul
    (stationary W1[:128]) since it broadcasts over k.
  - The masked K-sum is PSUM accumulation across the 48 per-k W3 matmuls;
    the b3*sum(mask) term opens the accumulation group as a rank-1 matmul.
  - LN stats are column sums via ones-matmuls; per-node mean/rstd rows are
    broadcast back over partitions with gpsimd partition_broadcast (keeps the
    residual/normalization path in full fp32).
  - fp32r (full-rate reduced-precision fp32) matmuls downstream of the edge
    path; h_E k-groups are cast fp32->fp16 during the SWDGE load DMA (KG=4
    adjacent k's per DMA for long contiguous runs and amortized descriptor
    emission).
"""

import os
import numpy as np
from contextlib import ExitStack

import concourse.bass as bass
import concourse.tile as tile
from concourse import bacc, mybir
from concourse.bass_utils import run_bass_kernel_spmd
from concourse.masks import make_identity

F32 = mybir.dt.float32
F32R = mybir.dt.float32r
F16 = mybir.dt.float16
BF16 = mybir.dt.bfloat16
AF = mybir.ActivationFunctionType

# Problem constants (hardcoded per contract)
B, N, K, H, NUM_IN = 4, 2048, 48, 128, 384
SCALE = 30.0
EPS = 1e-5
N_CORES = 8
NODES = B * N            # 8192
NPC = NODES // N_CORES   # 1024 nodes per core

EDGE = os.environ.get("EDGE", "fp16")
EDGE_DT = {"fp16": F16, "bf16": BF16, "fp32r": F32}[EDGE]


def _edge_np_dtype():
    import ml_dtypes
    return {"fp16": np.float16, "bf16": ml_dtypes.bfloat16,
            "fp32r": np.float32}[EDGE]


def build(alpha: float, masks_ones: bool, npc: int = NPC):
    # Note: the fp32r edge variant no longer fits the paired-transpose PSUM
    # budget; the shipping configurations are the 2-byte edge dtypes.
    assert EDGE_DT != F32, "EDGE=fp32r unsupported; use fp16 (default) or bf16"
    CH = 512                     # nodes per dh-accumulation chunk (= k-window width)
    assert npc % CH == 0
    n_chunks = npc // CH
    edge_is_f32 = EDGE_DT == F32
    edge_mm = F32R if edge_is_f32 else EDGE_DT   # matmul dtype on the edge path
    xt_dt = F32R if edge_is_f32 else EDGE_DT     # storage dtype of X^T
    tp_dt = F32 if edge_is_f32 else EDGE_DT      # transpose/psum dtype (f32r invalid there)

    nc = bacc.Bacc("TRN2", target_bir_lowering=False, debug=False, num_devices=N_CORES)

    # ---- DRAM I/O ----
    hE_d = nc.dram_tensor("hE", [npc, K, NUM_IN], F32, kind="ExternalInput").ap()
    hV_d = nc.dram_tensor("hV", [npc, H], F32, kind="ExternalInput").ap()
    w1v_d = nc.dram_tensor("w1v", [H, H], F32, kind="ExternalInput").ap()
    w1e_d = nc.dram_tensor("w1e", [NUM_IN, H],
                           F32 if edge_is_f32 else EDGE_DT, kind="ExternalInput").ap()
    w2_d = nc.dram_tensor("w2", [H, H], F32, kind="ExternalInput").ap()
    w3_d = nc.dram_tensor("w3", [H, H], F32, kind="ExternalInput").ap()
    wi_d = nc.dram_tensor("wi", [H, 4 * H], F32, kind="ExternalInput").ap()
    wo_d = nc.dram_tensor("wo", [4 * H, H], F32, kind="ExternalInput").ap()
    # rows: b1, b2, g1, be1, g2, be2, alpha*bo
    vecs_d = nc.dram_tensor("vecs", [7, H], F32, kind="ExternalInput").ap()
    bi_d = nc.dram_tensor("bi", [4 * H], F32, kind="ExternalInput").ap()
    b3_d = nc.dram_tensor("b3", [H], F32, kind="ExternalInput").ap()
    summask_d = nc.dram_tensor("summask", [npc], F32, kind="ExternalInput").ap()
    out_d = nc.dram_tensor("out", [npc, H], F32, kind="ExternalOutput").ap()
    if not masks_ones:
        mA_d = nc.dram_tensor("mA", [npc, K], F32, kind="ExternalInput").ap()
        mV_d = nc.dram_tensor("mV", [npc], F32, kind="ExternalInput").ap()

    with tile.TileContext(nc) as tc, ExitStack() as ctx:
        singles = ctx.enter_context(tc.tile_pool(name="singles", bufs=1))
        stage = ctx.enter_context(tc.tile_pool(name="stage", bufs=2))
        nat_pool = ctx.enter_context(
            tc.tile_pool(name="nat", bufs=12 if EDGE_DT != F32 else 6))
        xt_pool = ctx.enter_context(tc.tile_pool(name="xt", bufs=3))
        m_pool = ctx.enter_context(tc.tile_pool(name="m", bufs=3))
        fin_pool = ctx.enter_context(tc.tile_pool(name="fin", bufs=1))
        # PSUM: exactly one tag per pool; every tile is <= one bank.
        ps_tp = ctx.enter_context(tc.tile_pool(name="ps_tp", bufs=2, space="PSUM"))
        ps_a = ctx.enter_context(tc.tile_pool(name="ps_a", bufs=2, space="PSUM"))
        ps_b = ctx.enter_context(tc.tile_pool(name="ps_b", bufs=2, space="PSUM"))
        ps_dh = ctx.enter_context(tc.tile_pool(name="ps_dh", bufs=2, space="PSUM"))

        def tp_tile():
            # one fp16 bank holds a window-pair's transposes for one ftile
            return ps_tp.tile([128, 2 * CH], tp_dt, tag="tp", name="tp")

        def pa_tile(dt=F32):
            return ps_a.tile([128, CH], dt, tag="pa", name="pa")

        def pb_tile(dt=F32):
            return ps_b.tile([128, CH], dt, tag="pb", name="pb")

        # ---- constants / weights ----
        ident_e = singles.tile([128, 128], tp_dt)
        make_identity(nc, ident_e)
        if edge_is_f32:
            ident_f = ident_e
        else:
            ident_f = singles.tile([128, 128], F32)
            make_identity(nc, ident_f)

        def load_f32r(name, dram_ap, shape):
            t32 = stage.tile(shape, F32, tag="ldstage" + str(shape), name="ldstage")
            nc.sync.dma_start(t32[:], dram_ap)
            tr = singles.tile(shape, F32R, tag=name, name=name)
            nc.vector.tensor_copy(tr[:], t32[:])
            return tr

        w1v = load_f32r("w1v", w1v_d, [H, H])
        w2 = load_f32r("w2", w2_d, [H, H])
        w3 = load_f32r("w3", w3_d, [H, H])
        wi = load_f32r("wi", wi_d, [H, 4 * H])
        wo = load_f32r("wo", wo_d.rearrange("(j p) h -> p j h", p=H), [H, 4, H])
        b3row = load_f32r("b3row", b3_d.rearrange("(o h) -> o h", o=1), [1, H])
        if edge_is_f32:
            w1e = load_f32r("w1e", w1e_d.rearrange("(a p) h -> p a h", p=H),
                            [H, NUM_IN // H, H])
        else:
            w1e = singles.tile([H, NUM_IN // H, H], EDGE_DT)
            nc.sync.dma_start(w1e[:], w1e_d.rearrange("(a p) h -> p a h", p=H))

        vecs = singles.tile([H, 7], F32)
        nc.sync.dma_start(vecs[:], vecs_d.rearrange("v h -> h v"))
        bi_sb = singles.tile([H, 4], F32)
        nc.sync.dma_start(bi_sb[:], bi_d.rearrange("(j p) -> p j", p=H))
        b1, b2 = vecs[:, 0:1], vecs[:, 1:2]
        g1, be1 = vecs[:, 2:3], vecs[:, 3:4]
        g2, be2 = vecs[:, 4:5], vecs[:, 5:6]
        abo = vecs[:, 6:7]

        eps_c = singles.tile([1, 1], F32)
        nc.vector.memset(eps_c[:], EPS)
        ones_32 = singles.tile([H, H + 1], F32, name="ones_32")
        nc.vector.memset(ones_32[:], 1.0)
        ones_col = singles.tile([H, 1], F32R)
        nc.vector.tensor_copy(ones_col[:], ones_32[:, 0:1])
        ones_row = singles.tile([1, H], F32R)
        nc.vector.tensor_copy(ones_row[:], ones_32[0:1, 1:H + 1])

        summask_r = load_f32r("summask",
                              summask_d.rearrange("(o n) -> o n", o=1), [1, npc])

        # ---- h_V transposed: hVT [128 h, npc nodes] (fp32 master + f32r) ----
        hVT = singles.tile([H, npc], F32)
        for t in range(npc // 128):
            vnat = nat_pool.tile([128, H], F32, tag="vnat")
            nc.sync.dma_start(vnat[:], hV_d[t * 128:(t + 1) * 128, :])
            vps = ps_tp.tile([128, CH], F32, tag="tp", name="vps")
            nc.tensor.transpose(vps[:, 0:128], vnat[:], ident_f[:])
            nc.vector.tensor_copy(hVT[:, t * 128:(t + 1) * 128], vps[:, 0:128])
        hVT_r = singles.tile([H, npc], F32R)
        nc.vector.tensor_copy(hVT_r[:], hVT[:])

        if not masks_ones:
            mV_sb = singles.tile([1, npc], F32)
            nc.sync.dma_start(mV_sb[:], mV_d.rearrange("(o n) -> o n", o=1))

        x1_all = fin_pool.tile([H, npc], F32, tag="x1")

        n_ftiles = NUM_IN // 128  # 3
        rt = CH // 128            # 4 row-tiles per window

        KG = int(os.environ.get("KG", "4"))  # k's per load DMA (contiguous in DRAM)
        srcg = hE_d.rearrange("(c t p) (g k) f -> c g t p (k f)",
                              t=rt, p=128, k=KG)
        dh_tiles = []
        for c in range(n_chunks):
            dh_ps = ps_dh.tile([H, CH], F32, tag="dh")
            dh_tiles.append(dh_ps)
            # open the K-sum group with the b3 * summask rank-1 term
            nc.tensor.matmul(dh_ps[:], b3row[:],
                             summask_r[:, c * CH:(c + 1) * CH],
                             start=True, stop=False)
            for g in range(K // KG):
                # ---- load h_E k-group: nodes [c*CH,(c+1)*CH), k in [g*KG,(g+1)*KG) ----
                nats = []
                for t in range(rt):
                    natt = nat_pool.tile([128, KG * NUM_IN],
                                         EDGE_DT if not edge_is_f32 else F32,
                                         tag="nat", name="nat")
                    if edge_is_f32:
                        nc.sync.dma_start(natt[:], srcg[c, g, t])
                    else:
                        nc.gpsimd.dma_start(natt[:], srcg[c, g, t])  # casting DMA
                    nats.append(natt)
                for pair in range(KG // 2):
                    # ---- transpose both windows of the pair to xT ----
                    xT = xt_pool.tile([128, n_ftiles, 2, CH], xt_dt, tag="xT")
                    for ft in range(n_ftiles):
                        tps = tp_tile()
                        n_tp = 2 * rt
                        for i in range(n_tp):
                            kk, t = 2 * pair + i // rt, i % rt
                            nc.tensor.matmul(
                                tps[:, i * 128:(i + 1) * 128],
                                nats[t][:, kk * NUM_IN + ft * 128:
                                        kk * NUM_IN + (ft + 1) * 128],
                                ident_e[:],
                                is_transpose=True,
                                start=(i == 0), stop=(i == n_tp - 1),
                            )
                        nc.vector.tensor_copy(
                            xT[:, ft, :, :].rearrange("p a b -> p (a b)"), tps[:])
                    # ---- W1/W2 per window; W3 accumulates the K-sum ----
                    m2s = []
                    for kk in range(2):
                        m1_ps = pa_tile()
                        nc.tensor.matmul(m1_ps[:], w1v[:],
                                         hVT_r[:, c * CH:(c + 1) * CH],
                                         start=True, stop=False)
                        for ft in range(n_ftiles):
                            nc.tensor.matmul(m1_ps[:],
                                             w1e[:, ft, :].bitcast(edge_mm),
                                             xT[:, ft, kk, :].bitcast(edge_mm),
                                             start=False,
                                             stop=(ft == n_ftiles - 1))
                        m1 = m_pool.tile([H, CH], F32R, tag="m1", name="m1")
                        nc.scalar.activation(m1[:], m1_ps[:], AF.Gelu, bias=b1)
                        m2_ps = pb_tile()
                        nc.tensor.matmul(m2_ps[:], w2[:], m1[:],
                                         start=True, stop=True)
                        m2 = m_pool.tile([H, CH], F32R, tag="m2", name="m2")
                        nc.scalar.activation(m2[:], m2_ps[:], AF.Gelu, bias=b2)
                        m2s.append(m2)
                    for kk in range(2):
                        w = g * KG + 2 * pair + kk
                        m2k = m2s[kk][:]
                        if not masks_ones:
                            mrow = m_pool.tile([1, CH], F32, tag="mrow")
                            nc.gpsimd.dma_start(
                                mrow[:],
                                mA_d.rearrange("(c n) k -> c k n", c=n_chunks)[c, w]
                                .rearrange("(o n) -> o n", o=1))
                            mb = m_pool.tile([128, CH], F32, tag="mb")
                            nc.gpsimd.partition_broadcast(mb[:], mrow[:])
                            m2m = m_pool.tile([H, CH], F32R, tag="m2m")
                            nc.vector.tensor_mul(m2m[:], m2k, mb[:])
                            m2k = m2m[:]
                        # ---- W3 + K-sum accumulation ----
                        nc.tensor.matmul(dh_ps[:], w3[:], m2k,
                                         start=False, stop=(w == K - 1))
            # ---- residual 1 ----
            dh_s = m_pool.tile([H, CH], F32, tag="dhs")
            nc.scalar.activation(dh_s[:], dh_ps[:], AF.Copy, scale=alpha / SCALE)
            nc.vector.tensor_add(x1_all[:, c * CH:(c + 1) * CH], dh_s[:],
                                 hVT[:, c * CH:(c + 1) * CH])

        # ================= finale over all npc nodes =================
        def layernorm(x_sb, g_ap, be_ap, tag, extra_mul=None):
            y = fin_pool.tile([H, npc], F32, tag="lnout_" + tag)
            xr = fin_pool.tile([H, npc], F32R, tag="ln_xr")
            nc.vector.tensor_copy(xr[:], x_sb[:])
            sq = fin_pool.tile([H, npc], F32R, tag="ln_sq")
            nc.scalar.activation(sq[:], x_sb[:], AF.Square)
            for c0 in range(0, npc, CH):
                cs = slice(c0, c0 + CH)
                st_a = pa_tile()
                st_b = pa_tile()
                st_x = st_a[0:1, 0:CH]
                st_q = st_b[0:1, 0:CH]
                nc.tensor.matmul(st_x, ones_col[:], xr[:, cs],
                                 start=True, stop=True)
                nc.tensor.matmul(st_q, ones_col[:],
                                 sq[:, cs], start=True, stop=True)
                mu = fin_pool.tile([1, CH], F32, tag="mu")
                nc.vector.tensor_scalar_mul(mu[:], st_x, 1.0 / H)
                musq = fin_pool.tile([1, CH], F32, tag="musq")
                nc.scalar.activation(musq[:], st_x, AF.Square, scale=1.0 / H)
                var = fin_pool.tile([1, CH], F32, tag="var")
                nc.vector.tensor_scalar_mul(var[:], st_q, 1.0 / H)
                nc.vector.tensor_sub(var[:], var[:], musq[:])
                sd = fin_pool.tile([1, CH], F32, tag="sd")
                nc.scalar.activation(sd[:], var[:], AF.Sqrt, bias=eps_c[:])
                rstd = fin_pool.tile([1, CH], F32, tag="rstd")
                nc.vector.reciprocal(rstd[:], sd[:])
                mu_b = fin_pool.tile([H, CH], F32, tag="mu_b")
                rstd_b = fin_pool.tile([H, CH], F32, tag="rstd_b")
                nc.gpsimd.partition_broadcast(mu_b[:], mu[:])
                nc.gpsimd.partition_broadcast(rstd_b[:], rstd[:])
                xc = fin_pool.tile([H, CH], F32, tag="xc")
                nc.vector.tensor_sub(xc[:], x_sb[:, cs], mu_b[:])
                xn = fin_pool.tile([H, CH], F32, tag="xn")
                nc.vector.tensor_mul(xn[:], xc[:], rstd_b[:])
                if extra_mul is None:
                    nc.scalar.activation(y[:, cs], xn[:], AF.Identity,
                                         bias=be_ap, scale=g_ap)
                else:
                    yt = fin_pool.tile([H, CH], F32, tag="yt")
                    nc.scalar.activation(yt[:], xn[:], AF.Identity,
                                         bias=be_ap, scale=g_ap)
                    nc.vector.tensor_mul(y[:, cs], yt[:], extra_mul[:, cs])
            return y

        h1 = layernorm(x1_all, g1, be1, "h1")
        h1r = fin_pool.tile([H, npc], F32R, tag="h1r")
        nc.vector.tensor_copy(h1r[:], h1[:])

        x2_all = fin_pool.tile([H, npc], F32, tag="x2")
        for c0 in range(0, npc, CH):
            cs = slice(c0, c0 + CH)
            u = fin_pool.tile([H, 4, CH], F32R, tag="u")
            for j in range(4):
                u_ps = pa_tile()
                nc.tensor.matmul(u_ps[:], wi[:, j * H:(j + 1) * H], h1r[:, cs],
                                 start=True, stop=True)
                nc.scalar.activation(u[:, j, :], u_ps[:],
                                     AF.Gelu, bias=bi_sb[:, j:j + 1])
            d2_ps = pb_tile()[:, 0:CH]
            for j in range(4):
                nc.tensor.matmul(d2_ps, wo[:, j, :], u[:, j, :],
                                 start=(j == 0), stop=(j == 3))
            d2 = fin_pool.tile([H, CH], F32, tag="d2s")
            # x2 = alpha*(d2_ps + bo) + h1 ; abo = alpha*bo precomputed on host
            nc.scalar.activation(d2[:], d2_ps, AF.Identity, scale=alpha, bias=abo)
            nc.vector.tensor_add(x2_all[:, cs], d2[:], h1[:, cs])

        mvb = None
        if not masks_ones:
            mvb = fin_pool.tile([128, npc], F32, tag="mvb")
            nc.gpsimd.partition_broadcast(mvb[:], mV_sb[:])
        y = layernorm(x2_all, g2, be2, "y", extra_mul=mvb)

        for t in range(npc // 128):
            ops = ps_tp.tile([128, CH], F32, tag="tp", name="ops")
            nc.tensor.transpose(ops[:, 0:128], y[:, t * 128:(t + 1) * 128], ident_f[:])
            onat = nat_pool.tile([128, H], F32, tag="onat")
            nc.vector.tensor_copy(onat[:], ops[:, 0:128])
            nc.sync.dma_start(out_d[t * 128:(t + 1) * 128, :], onat[:])

    nc.compile()
    return nc


_CACHE = {}
LAST_RESULT = None


def _get_nc(alpha: float, masks_ones: bool):
    key = (alpha, masks_ones, EDGE)
    if key not in _CACHE:
        _CACHE[key] = build(alpha, masks_ones)
    return _CACHE[key]


def kernel(h_V, h_E, mask_V, mask_attend, W1, b1, W2, b2, W3, b3,
           g1, be1, g2, be2, Wi, bi, Wo, bo, alpha):
    h_V = np.asarray(h_V, dtype=np.float32)
    h_E = np.ascontiguousarray(np.asarray(h_E, dtype=np.float32))
    mask_V = np.asarray(mask_V, dtype=np.float32)
    mask_attend = np.asarray(mask_attend, dtype=np.float32)
    alpha_f = float(np.asarray(alpha).reshape(-1)[0])
    masks_ones = bool(np.all(mask_attend == 1.0) and np.all(mask_V == 1.0))

    nc = _get_nc(alpha_f, masks_ones)

    w1 = np.asarray(W1, np.float32)
    w1v = np.ascontiguousarray(w1[:H])
    w1e = np.ascontiguousarray(w1[H:]).astype(_edge_np_dtype())
    vecs = np.stack([np.asarray(v, np.float32).reshape(H) for v in
                     (b1, b2, g1, be1, g2, be2)] +
                    [alpha_f * np.asarray(bo, np.float32).reshape(H)])

    hE_flat = h_E.reshape(NODES, K, NUM_IN)
    hV_flat = np.ascontiguousarray(h_V.reshape(NODES, H))
    mA_flat = np.ascontiguousarray(mask_attend.reshape(NODES, K))
    mV_flat = np.ascontiguousarray(mask_V.reshape(NODES))
    summask = np.ascontiguousarray(mA_flat.sum(axis=1).astype(np.float32))

    shared = {
        "w1v": w1v, "w1e": w1e,
        "w2": np.ascontiguousarray(np.asarray(W2, np.float32)),
        "w3": np.ascontiguousarray(np.asarray(W3, np.float32)),
        "wi": np.ascontiguousarray(np.asarray(Wi, np.float32)),
        "wo": np.ascontiguousarray(np.asarray(Wo, np.float32)),
        "vecs": np.ascontiguousarray(vecs),
        "bi": np.ascontiguousarray(np.asarray(bi, np.float32).reshape(4 * H)),
        "b3": np.ascontiguousarray(np.asarray(b3, np.float32).reshape(H)),
    }
    in_maps = []
    for c in range(N_CORES):
        s = slice(c * NPC, (c + 1) * NPC)
        m = dict(shared)
        m["hE"] = hE_flat[s]
        m["hV"] = hV_flat[s]
        m["summask"] = summask[s]
        if not masks_ones:
            m["mA"] = mA_flat[s]
            m["mV"] = mV_flat[s]
        in_maps.append(m)

    trace = os.environ.get("KERNEL_TRACE", "0") == "1"
    r = run_bass_kernel_spmd(nc, in_maps, list(range(N_CORES)), trace=trace)
    global LAST_RESULT
    LAST_RESULT = r
    out = np.concatenate([r.results[c]["out"] for c in range(N_CORES)], axis=0)
    return out.reshape(B, N, H).astype(np.float32)

